# revision 3
# baseline (speedup 1.0000x reference)
"""Masked ("sparse") attention with shared QK projection on 8 TRN2 NeuronCores.

Reference computation (per batch b):
    qp = q @ w_q.T                       [NQ, E]
    kp = k @ w_k.T                       [NK, E]
    S  = (qp @ kp.T) * E**-0.5           [NQ, NK]
    S[m masked] = -inf ; P = softmax(S, axis=-1)
    x  = P @ kp                          [NQ, E]

Device strategy (data-parallel over batch, 4 batches per core):
  * Host folds W = (w_q.T @ w_k) * E**-0.5 so that S = q @ W @ k.T.
  * Sparsity: masked keys contribute nothing (their softmax weight is 0),
    so the key axis is COMPACTED per batch to ceil(m_eff/128)*128 columns
    via an indirect-DMA row gather of k. Batches are assigned to cores by
    descending m_eff so every core runs the same per-slot block shape
    (for the fixed harness seed: (5,5,4,4) 128-blocks = 18 per core vs
    20 for a uniform 640 pad). Pad rows point at the batch's k row 0 and
    are killed by an additive -30000 bias on the exp.
  * q and k are staged in DRAM as bf16 (host cast); qT/kT are produced by
    the DMA XBAR transpose unit (dma_start_transpose), so the PE runs no
    transposes at all. The output is written bf16 and upcast on host.
  * The score matrix is built TRANSPOSED, S^T [m, n]: the additive key
    mask becomes a per-partition activation bias, exp needs no row-max
    (logits are O(5), masked rows underflow to exactly 0), and the exp
    output is already in the [m, n] layout the x-matmul contraction
    needs, so no P transposes either.
  * Per batch slot the device computes (contractions on TensorE, bf16):
        kc  = gather(k, idx)              [M_s, D]
        kT  = xbar-T(kc)                  [D, M_s]
        qT  = xbar-T(q)                   [D, NQ]
        G   = W @ kT                      [D, M_s]   (lhsT = W.T)
        kp  = kT.T @ w_k.T                [M_s, E]
        S^T = G.T @ qT  (per m-tile)      [M_s, NQ]
        PT  = exp(S^T + maskcol)          [M_s, NQ]  (no max needed)
        den = PT.T @ 1  (N=1 matmuls)     [NQ, 1]
        x   = (PT.T @ kp) * (1/den)       [NQ, E]
"""

import sys

sys.path.insert(0, "/opt/trn_rl_repo")

from contextlib import ExitStack

import numpy as np
import ml_dtypes

import concourse.bass as bass
import concourse.tile as tile
from concourse import bacc, mybir
from concourse.bass_utils import run_bass_kernel_spmd

B, NQ, NK = 32, 1024, 1024
D = E = 1024
N_CORES = 8
B_LOC = B // N_CORES

P = 128  # partition width
NB = NQ // P  # 128-blocks along a 1024 dim (=8)
MASK_NEG = -30000.0

COMPUTE_DT = mybir.dt.bfloat16
COMPUTE_NP = ml_dtypes.bfloat16

E_CHUNKS = [(0, 512), (512, 512)]  # chunks of a 1024 free dim, 1 PSUM bank each


def build_kernel_body(ctx, tc, outs, ins, slot_nmb):
    nc = tc.nc
    qb_d = ins["qb"]  # [B_LOC, NQ, D] bf16
    kb_d = ins["kb"]  # [B_LOC*NK, D] bf16 (gather target)
    wt_d = ins["wt"]  # [D, D] = W.T  (bf16)
    wkt_d = ins["wkt"]  # [D, E] = w_k.T (bf16)
    mb_d = ins["maskcol"]  # [P, NMB_TOT] f32: exp bias column per m-tile
    idx_d = ins["idx"]  # [P, NMB_TOT] int32: row p of (slot,group) -> kb row
    out_d = outs["out"]

    nmb_tot = sum(slot_nmb)
    nmb_max = max(slot_nmb)
    slot_base = [sum(slot_nmb[:s]) for s in range(len(slot_nmb))]

    const = ctx.enter_context(tc.tile_pool(name="const", bufs=1))
    kc_p = ctx.enter_context(tc.tile_pool(name="kc", bufs=nmb_max + 2))
    kT_p = ctx.enter_context(tc.tile_pool(name="kT", bufs=2 * NB))
    qT_p = ctx.enter_context(tc.tile_pool(name="qT", bufs=2 * NB))
    G_p = ctx.enter_context(tc.tile_pool(name="G", bufs=2 * NB))
    kp_p = ctx.enter_context(tc.tile_pool(name="kp", bufs=2 * nmb_max))
    PT_p = ctx.enter_context(tc.tile_pool(name="PT", bufs=2 * nmb_max))
    x_p = ctx.enter_context(tc.tile_pool(name="x", bufs=4))
    st_p = ctx.enter_context(tc.tile_pool(name="stats", bufs=2 * NB))
    ps_mm = ctx.enter_context(tc.tile_pool(name="ps_mm", bufs=3, space="PSUM"))
    ps_dn = ctx.enter_context(tc.tile_pool(name="ps_dn", bufs=2, space="PSUM"))

    # tiny/control inputs first on the sync queue so gathers can start
    idx_sb = const.tile([P, nmb_tot], mybir.dt.int32, tag="idx")
    nc.sync.dma_start(out=idx_sb, in_=idx_d)
    maskb = const.tile([P, nmb_tot], mybir.dt.float32, tag="maskb")
    nc.sync.dma_start(out=maskb, in_=mb_d)

    ones = const.tile([P, 1], COMPUTE_DT, tag="ones_col")
    nc.gpsimd.memset(ones, 1.0)

    # resident weights: WT as 8 [128(b), D(a)] tiles; WKT as 8 [128(d), E]
    wt_sb = []
    wkt_sb = []
    for i in range(NB):
        t = const.tile([P, D], COMPUTE_DT, tag=f"wt_sb{i}")
        nc.sync.dma_start(out=t, in_=wt_d[i * P : (i + 1) * P, :])
        wt_sb.append(t)
        t2 = const.tile([P, E], COMPUTE_DT, tag=f"wkt_sb{i}")
        nc.sync.dma_start(out=t2, in_=wkt_d[i * P : (i + 1) * P, :])
        wkt_sb.append(t2)

    for s, nmb in enumerate(slot_nmb):
        M_s = nmb * P
        base = slot_base[s]

        # ---- k-side: gather bf16 rows, XBAR-transpose into kT ----
        kT = [
            kT_p.tile([P, nmb_max * P], COMPUTE_DT, tag="kT", name=f"kT{dj}")
            for dj in range(NB)
        ]
        for g in range(nmb):
            kc = kc_p.tile([P, D], COMPUTE_DT, tag="kc")
            nc.gpsimd.indirect_dma_start(
                out=kc,
                out_offset=None,
                in_=kb_d,
                in_offset=bass.IndirectOffsetOnAxis(
                    ap=idx_sb[:, base + g : base + g + 1], axis=0
                ),
            )
            for dj in range(NB):
                nc.scalar.dma_start_transpose(
                    out=kT[dj][:, g * P : (g + 1) * P],
                    in_=kc[:, dj * P : (dj + 1) * P],
                )

        # ---- q-side: XBAR-transpose straight from DRAM bf16 ----
        qT = []
        for dj in range(NB):
            t = qT_p.tile([P, NQ], COMPUTE_DT, tag="qT")
            nc.scalar.dma_start_transpose(out=t, in_=qb_d[s, :, dj * P : (dj + 1) * P])
            qT.append(t)

        m_chunks = [(0, 512), (512, M_s - 512)] if M_s > 512 else [(0, M_s)]

        # ---- G = W @ kT : 8 x [128(a), M_s(m)] ----
        G = []
        for dj in range(NB):
            ps = ps_mm.tile([P, NB * P], mybir.dt.float32, tag="ps_mm")
            for c0, cw in m_chunks:
                for di in range(NB):
                    nc.tensor.matmul(
                        ps[:, c0 : c0 + cw],
                        wt_sb[di][:, dj * P : (dj + 1) * P],
                        kT[di][:, c0 : c0 + cw],
                        start=(di == 0),
                        stop=(di == NB - 1),
                    )
            t = G_p.tile([P, nmb_max * P], COMPUTE_DT, tag="G")
            nc.vector.tensor_copy(out=t[:, :M_s], in_=ps[:, :M_s])
            G.append(t)

        # ---- kp = kT.T @ wkT : nmb x [128(m), 1024(e)] ----
        kp = []
        for mi in range(nmb):
            ps = ps_mm.tile([P, NB * P], mybir.dt.float32, tag="ps_mm")
            for c0, cw in E_CHUNKS:
                for di in range(NB):
                    nc.tensor.matmul(
                        ps[:, c0 : c0 + cw],
                        kT[di][:, mi * P : (mi + 1) * P],
                        wkt_sb[di][:, c0 : c0 + cw],
                        start=(di == 0),
                        stop=(di == NB - 1),
                    )
            t = kp_p.tile([P, E], COMPUTE_DT, tag="kp")
            nc.vector.tensor_copy(out=t, in_=ps)
            kp.append(t)

        # ---- S^T = G.T @ qT  then  PT = exp(S^T + maskcol) ----
        PT = []
        for mi in range(nmb):
            ps = ps_mm.tile([P, NB * P], mybir.dt.float32, tag="ps_mm")
            for c0, cw in E_CHUNKS:
                for dj in range(NB):
                    nc.tensor.matmul(
                        ps[:, c0 : c0 + cw],
                        G[dj][:, mi * P : (mi + 1) * P],
                        qT[dj][:, c0 : c0 + cw],
                        start=(dj == 0),
                        stop=(dj == NB - 1),
                    )
            pt = PT_p.tile([P, NB * P], COMPUTE_DT, tag="PT")
            nc.scalar.activation(
                out=pt,
                in_=ps,
                func=mybir.ActivationFunctionType.Exp,
                bias=maskb[:, base + mi : base + mi + 1],
                scale=1.0,
            )
            PT.append(pt)

        # ---- denom[n] = sum_m PT[m, n] via N=1 matmuls; recip ----
        # ---- x = (PT.T @ kp) / denom ----
        for ni in range(NB):
            dn = ps_dn.tile([P, 1], mybir.dt.float32, tag="ps_dn")
            ps = ps_mm.tile([P, NB * P], mybir.dt.float32, tag="ps_mm")
            for mi in range(nmb):
                lhsT = PT[mi][:, ni * P : (ni + 1) * P]
                nc.tensor.matmul(
                    dn,
                    lhsT,
                    ones,
                    start=(mi == 0),
                    stop=(mi == nmb - 1),
                )
                for c0, cw in E_CHUNKS:
                    nc.tensor.matmul(
                        ps[:, c0 : c0 + cw],
                        lhsT,
                        kp[mi][:, c0 : c0 + cw],
                        start=(mi == 0),
                        stop=(mi == nmb - 1),
                    )
            rec = st_p.tile([P, 1], mybir.dt.float32, tag="recip")
            nc.vector.reciprocal(rec, dn)
            xt = x_p.tile([P, E], COMPUTE_DT, tag="x")
            nc.vector.tensor_scalar_mul(xt, ps, rec)
            nc.sync.dma_start(out=out_d[s, ni * P : (ni + 1) * P, :], in_=xt)


def build_module(slot_nmb):
    nc = bacc.Bacc("TRN2", target_bir_lowering=False, debug=False)
    b_loc = len(slot_nmb)
    nmb_tot = sum(slot_nmb)
    ins = {
        "qb": nc.dram_tensor(
            "qb", [b_loc, NQ, D], COMPUTE_DT, kind="ExternalInput"
        ).ap(),
        "kb": nc.dram_tensor(
            "kb", [b_loc * NK, D], COMPUTE_DT, kind="ExternalInput"
        ).ap(),
        "wt": nc.dram_tensor("wt", [D, D], COMPUTE_DT, kind="ExternalInput").ap(),
        "wkt": nc.dram_tensor("wkt", [D, E], COMPUTE_DT, kind="ExternalInput").ap(),
        "maskcol": nc.dram_tensor(
            "maskcol", [P, nmb_tot], mybir.dt.float32, kind="ExternalInput"
        ).ap(),
        "idx": nc.dram_tensor(
            "idx", [P, nmb_tot], mybir.dt.int32, kind="ExternalInput"
        ).ap(),
    }
    outs = {
        "out": nc.dram_tensor(
            "out", [b_loc, NQ, E], COMPUTE_DT, kind="ExternalOutput"
        ).ap()
    }
    with tile.TileContext(nc) as tc:
        with ExitStack() as ctx:
            build_kernel_body(ctx, tc, outs, ins, slot_nmb)
    nc.compile()
    return nc


def host_prep(q, k, attn_mask, w_q, w_k, n_cores=N_CORES):
    """Weight folding, batch->core assignment, gather indices, input maps."""
    scale = float(E) ** -0.5
    W = (w_q.astype(np.float64).T @ w_k.astype(np.float64)) * scale
    wt = np.ascontiguousarray(W.T).astype(COMPUTE_NP)
    wkt = np.ascontiguousarray(w_k.T).astype(COMPUTE_NP)
    qbf = q.astype(COMPUTE_NP)
    kbf = k.astype(COMPUTE_NP)

    bsz = q.shape[0]
    b_loc = bsz // n_cores
    m_eff = (attn_mask != 0).sum(axis=1)
    order = np.argsort(-m_eff, kind="stable")  # descending m_eff
    # slot s of core c runs batch order[s*n_cores + c]; slot shape is the
    # max block count in each slot group = its first (largest) member.
    blocks = np.maximum(np.ceil(m_eff / P).astype(int), 1)
    slot_nmb = tuple(int(blocks[order[s * n_cores]]) for s in range(b_loc))
    nmb_tot = sum(slot_nmb)
    slot_base = [sum(slot_nmb[:s]) for s in range(b_loc)]

    in_maps = []
    perm = np.zeros((n_cores, b_loc), np.int64)
    for c in range(n_cores):
        idx = np.zeros((P, nmb_tot), np.int32)
        maskcol = np.full((nmb_tot, P), np.float32(MASK_NEG), np.float32)
        qs = []
        ks = []
        for s in range(b_loc):
            gb = int(order[s * n_cores + c])
            perm[c, s] = gb
            qs.append(qbf[gb])
            ks.append(kbf[gb])
            rows = np.nonzero(attn_mask[gb])[0].astype(np.int64)
            m_pad = slot_nmb[s] * P
            assert len(rows) <= m_pad, (gb, len(rows), m_pad)
            padded = np.zeros(m_pad, np.int64)
            padded[: len(rows)] = rows
            idx[:, slot_base[s] : slot_base[s] + slot_nmb[s]] = (
                (padded + s * NK).reshape(slot_nmb[s], P).T
            )
            maskcol.reshape(-1)[slot_base[s] * P : slot_base[s] * P + len(rows)] = 0.0
        in_maps.append(
            {
                "qb": np.ascontiguousarray(np.stack(qs)),
                "kb": np.ascontiguousarray(np.concatenate(ks, axis=0)),
                "wt": wt,
                "wkt": wkt,
                "maskcol": np.ascontiguousarray(maskcol.T),
                "idx": idx,
            }
        )
    return in_maps, perm, slot_nmb


_NC_CACHE = {}


def kernel(q, k, attn_mask, w_q, w_k, trace=False):
    q = np.asarray(q, dtype=np.float32)
    k = np.asarray(k, dtype=np.float32)
    w_q = np.asarray(w_q, dtype=np.float32)
    w_k = np.asarray(w_k, dtype=np.float32)
    attn_mask = np.asarray(attn_mask)

    in_maps, perm, slot_nmb = host_prep(q, k, attn_mask, w_q, w_k)
    if slot_nmb not in _NC_CACHE:
        _NC_CACHE[slot_nmb] = build_module(slot_nmb)
    nc = _NC_CACHE[slot_nmb]

    res = run_bass_kernel_spmd(nc, in_maps, core_ids=list(range(N_CORES)), trace=trace)
    out = np.zeros((B, NQ, E), np.float32)
    for c in range(N_CORES):
        out[perm[c]] = res.results[c]["out"].astype(np.float32)
    if trace:
        kernel.last_exec_time_ns = res.exec_time_ns
        kernel.last_results = res
    return out


# revision 7
# speedup vs baseline: 1.6939x; 1.6939x over previous
"""Masked ("sparse") attention with shared QK projection on 8 TRN2 NeuronCores.

Reference computation (per batch b):
    qp = q @ w_q.T                       [NQ, E]
    kp = k @ w_k.T                       [NK, E]
    S  = (qp @ kp.T) * E**-0.5           [NQ, NK]
    S[m masked] = -inf ; P = softmax(S, axis=-1)
    x  = P @ kp                          [NQ, E]

Device strategy (data-parallel over batch, 4 batches per core):
  * Host folds W = (w_q.T @ w_k) * E**-0.5 so that S = q @ W @ k.T.
  * Sparsity: masked keys contribute nothing (their softmax weight is 0),
    so the key axis is COMPACTED per batch to ceil(m_eff/128)*128 columns
    via an indirect-DMA row gather of k. Batches are assigned to cores by
    descending m_eff so every core runs the same per-slot block shape
    (for the fixed harness seed: (5,5,4,4) 128-blocks = 18 per core vs
    20 for a uniform 640 pad). Pad rows point at the batch's k row 0 and
    are killed by an additive -30000 bias on the exp.
  * q and k are staged in DRAM as bf16 (host cast); qT/kT are produced by
    the DMA XBAR transpose unit (dma_start_transpose), so the PE runs no
    transposes at all. The output is written bf16 and upcast on host.
  * The score matrix is built TRANSPOSED, S^T [m, n]: the additive key
    mask becomes a per-partition activation bias, exp needs no row-max
    (logits are O(5), masked rows underflow to exactly 0), and the exp
    output is already in the [m, n] layout the x-matmul contraction
    needs, so no P transposes either.
  * Per batch slot the device computes (contractions on TensorE, bf16):
        kc  = gather(k, idx)              [M_s, D]
        kT  = xbar-T(kc)                  [D, M_s]
        qT  = xbar-T(q)                   [D, NQ]
        G   = W @ kT                      [D, M_s]   (lhsT = W.T)
        kp  = kT.T @ w_k.T                [M_s, E]
        S^T = G.T @ qT  (per m-tile)      [M_s, NQ]
        PT  = exp(S^T + maskcol)          [M_s, NQ]  (no max needed)
        den = PT.T @ 1  (N=1 matmuls)     [NQ, 1]
        x   = (PT.T @ kp) * (1/den)       [NQ, E]
"""

import sys

sys.path.insert(0, "/opt/trn_rl_repo")

from contextlib import ExitStack

import numpy as np
import ml_dtypes

import concourse.bass as bass
import concourse.tile as tile
from concourse import bacc, mybir
from concourse.bass_utils import run_bass_kernel_spmd
from concourse.masks import make_identity

B, NQ, NK = 32, 1024, 1024
D = E = 1024
N_CORES = 8
B_LOC = B // N_CORES

P = 128  # partition width
NB = NQ // P  # 128-blocks along a 1024 dim (=8)
MASK_NEG = -30000.0

COMPUTE_DT = mybir.dt.bfloat16
COMPUTE_NP = ml_dtypes.bfloat16

E_CHUNKS = [(0, 512), (512, 512)]  # chunks of a 1024 free dim, 1 PSUM bank each


def build_kernel_body(ctx, tc, outs, ins, slot_nmb):
    nc = tc.nc
    qb_d = ins["qb"]  # [B_LOC, NQ, D] bf16
    kb_d = ins["kb"]  # [B_LOC*NK, D] bf16 (gather target)
    wt_d = ins["wt"]  # [D, D] = W.T  (bf16)
    wkt_d = ins["wkt"]  # [D, E] = w_k.T (bf16)
    mb_d = ins["maskcol"]  # [P, NMB_TOT] f32: exp bias column per m-tile
    idx_d = ins["idx"]  # [P, NMB_TOT] int32: row p of (slot,group) -> kb row
    out_d = outs["out"]

    nmb_tot = sum(slot_nmb)
    nmb_max = max(slot_nmb)
    slot_base = [sum(slot_nmb[:s]) for s in range(len(slot_nmb))]

    const = ctx.enter_context(tc.tile_pool(name="const", bufs=1))
    kc_p = ctx.enter_context(tc.tile_pool(name="kc", bufs=nmb_max + 2))
    kT_p = ctx.enter_context(tc.tile_pool(name="kT", bufs=2 * NB))
    qT_p = ctx.enter_context(tc.tile_pool(name="qT", bufs=2 * NB))
    G_p = ctx.enter_context(tc.tile_pool(name="G", bufs=2 * NB))
    kp_p = ctx.enter_context(tc.tile_pool(name="kp", bufs=2 * nmb_max))
    PT_p = ctx.enter_context(tc.tile_pool(name="PT", bufs=2 * nmb_max))
    x_p = ctx.enter_context(tc.tile_pool(name="x", bufs=4))
    st_p = ctx.enter_context(tc.tile_pool(name="stats", bufs=2 * NB))
    ps_mm = ctx.enter_context(tc.tile_pool(name="ps_mm", bufs=2, space="PSUM"))
    ps_tp = ctx.enter_context(tc.tile_pool(name="ps_tp", bufs=2, space="PSUM"))
    ps_dn = ctx.enter_context(tc.tile_pool(name="ps_dn", bufs=2, space="PSUM"))

    # tiny/control inputs first on the sync queue so gathers can start
    idx_sb = const.tile([P, nmb_tot], mybir.dt.int32, tag="idx")
    nc.sync.dma_start(out=idx_sb, in_=idx_d)
    maskb = const.tile([P, nmb_tot], mybir.dt.float32, tag="maskb")
    nc.sync.dma_start(out=maskb, in_=mb_d)

    ones = const.tile([P, 1], COMPUTE_DT, tag="ones_col")
    nc.gpsimd.memset(ones, 1.0)
    ident = const.tile([P, P], COMPUTE_DT, tag="ident")
    make_identity(nc, ident)

    # resident weights: WT as 8 [128(b), D(a)] tiles; WKT as 8 [128(d), E]
    wt_sb = []
    wkt_sb = []
    for i in range(NB):
        t = const.tile([P, D], COMPUTE_DT, tag=f"wt_sb{i}")
        nc.sync.dma_start(out=t, in_=wt_d[i * P : (i + 1) * P, :])
        wt_sb.append(t)
        t2 = const.tile([P, E], COMPUTE_DT, tag=f"wkt_sb{i}")
        nc.sync.dma_start(out=t2, in_=wkt_d[i * P : (i + 1) * P, :])
        wkt_sb.append(t2)

    for s, nmb in enumerate(slot_nmb):
        M_s = nmb * P
        base = slot_base[s]

        # ---- k-side: gather bf16 rows, PE-transpose into kT ----
        kc = []
        for g in range(nmb):
            t = kc_p.tile([P, D], COMPUTE_DT, tag="kc", name=f"kc{g}")
            nc.gpsimd.indirect_dma_start(
                out=t,
                out_offset=None,
                in_=kb_d,
                in_offset=bass.IndirectOffsetOnAxis(
                    ap=idx_sb[:, base + g : base + g + 1], axis=0
                ),
            )
            kc.append(t)
        kT = []
        for dj in range(NB):
            ps = ps_tp.tile([P, nmb_max * P], COMPUTE_DT, tag="ps_tp")
            for g in range(nmb):
                nc.tensor.transpose(
                    ps[:, g * P : (g + 1) * P],
                    kc[g][:, dj * P : (dj + 1) * P],
                    ident,
                )
            t = kT_p.tile([P, nmb_max * P], COMPUTE_DT, tag="kT", name=f"kT{dj}")
            nc.scalar.copy(out=t[:, :M_s], in_=ps[:, :M_s])
            kT.append(t)

        # ---- q-side: XBAR-transpose straight from DRAM bf16 ----
        qT = []
        for dj in range(NB):
            t = qT_p.tile([P, NQ], COMPUTE_DT, tag="qT")
            nc.sync.dma_start_transpose(out=t, in_=qb_d[s, :, dj * P : (dj + 1) * P])
            qT.append(t)

        m_chunks = [(0, 512), (512, M_s - 512)] if M_s > 512 else [(0, M_s)]

        # ---- G = W @ kT : 8 x [128(a), M_s(m)] ----
        G = []
        for dj in range(NB):
            ps = ps_mm.tile([P, NB * P], mybir.dt.float32, tag="ps_mm")
            for c0, cw in m_chunks:
                for di in range(NB):
                    nc.tensor.matmul(
                        ps[:, c0 : c0 + cw],
                        wt_sb[di][:, dj * P : (dj + 1) * P],
                        kT[di][:, c0 : c0 + cw],
                        start=(di == 0),
                        stop=(di == NB - 1),
                    )
            t = G_p.tile([P, nmb_max * P], COMPUTE_DT, tag="G")
            nc.vector.tensor_copy(out=t[:, :M_s], in_=ps[:, :M_s])
            G.append(t)

        # ---- kp = kT.T @ wkT : nmb x [128(m), 1024(e)] ----
        kp = []
        for mi in range(nmb):
            ps = ps_mm.tile([P, NB * P], mybir.dt.float32, tag="ps_mm")
            for c0, cw in E_CHUNKS:
                for di in range(NB):
                    nc.tensor.matmul(
                        ps[:, c0 : c0 + cw],
                        kT[di][:, mi * P : (mi + 1) * P],
                        wkt_sb[di][:, c0 : c0 + cw],
                        start=(di == 0),
                        stop=(di == NB - 1),
                    )
            t = kp_p.tile([P, E], COMPUTE_DT, tag="kp")
            nc.vector.tensor_copy(out=t, in_=ps)
            kp.append(t)

        # ---- S^T = G.T @ qT  then  PT = exp(S^T + maskcol) ----
        PT = []
        for mi in range(nmb):
            ps = ps_mm.tile([P, NB * P], mybir.dt.float32, tag="ps_mm")
            for c0, cw in E_CHUNKS:
                for dj in range(NB):
                    nc.tensor.matmul(
                        ps[:, c0 : c0 + cw],
                        G[dj][:, mi * P : (mi + 1) * P],
                        qT[dj][:, c0 : c0 + cw],
                        start=(dj == 0),
                        stop=(dj == NB - 1),
                    )
            pt = PT_p.tile([P, NB * P], COMPUTE_DT, tag="PT")
            nc.scalar.activation(
                out=pt,
                in_=ps,
                func=mybir.ActivationFunctionType.Exp,
                bias=maskb[:, base + mi : base + mi + 1],
                scale=1.0,
            )
            PT.append(pt)

        # ---- denom[n] = sum_m PT[m, n] via N=1 matmuls; recip ----
        # ---- x = (PT.T @ kp) / denom ----
        for ni in range(NB):
            dn = ps_dn.tile([P, 1], mybir.dt.float32, tag="ps_dn")
            ps = ps_mm.tile([P, NB * P], mybir.dt.float32, tag="ps_mm")
            for mi in range(nmb):
                lhsT = PT[mi][:, ni * P : (ni + 1) * P]
                nc.tensor.matmul(
                    dn,
                    lhsT,
                    ones,
                    start=(mi == 0),
                    stop=(mi == nmb - 1),
                )
                for c0, cw in E_CHUNKS:
                    nc.tensor.matmul(
                        ps[:, c0 : c0 + cw],
                        lhsT,
                        kp[mi][:, c0 : c0 + cw],
                        start=(mi == 0),
                        stop=(mi == nmb - 1),
                    )
            rec = st_p.tile([P, 1], mybir.dt.float32, tag="recip")
            nc.vector.reciprocal(rec, dn)
            xt = x_p.tile([P, E], COMPUTE_DT, tag="x")
            nc.vector.tensor_scalar_mul(xt, ps, rec)
            nc.sync.dma_start(out=out_d[s, ni * P : (ni + 1) * P, :], in_=xt)


def build_module(slot_nmb):
    nc = bacc.Bacc("TRN2", target_bir_lowering=False, debug=False)
    b_loc = len(slot_nmb)
    nmb_tot = sum(slot_nmb)
    ins = {
        "qb": nc.dram_tensor(
            "qb", [b_loc, NQ, D], COMPUTE_DT, kind="ExternalInput"
        ).ap(),
        "kb": nc.dram_tensor(
            "kb", [b_loc * NK, D], COMPUTE_DT, kind="ExternalInput"
        ).ap(),
        "wt": nc.dram_tensor("wt", [D, D], COMPUTE_DT, kind="ExternalInput").ap(),
        "wkt": nc.dram_tensor("wkt", [D, E], COMPUTE_DT, kind="ExternalInput").ap(),
        "maskcol": nc.dram_tensor(
            "maskcol", [P, nmb_tot], mybir.dt.float32, kind="ExternalInput"
        ).ap(),
        "idx": nc.dram_tensor(
            "idx", [P, nmb_tot], mybir.dt.int32, kind="ExternalInput"
        ).ap(),
    }
    outs = {
        "out": nc.dram_tensor(
            "out", [b_loc, NQ, E], COMPUTE_DT, kind="ExternalOutput"
        ).ap()
    }
    with tile.TileContext(nc) as tc:
        with ExitStack() as ctx:
            build_kernel_body(ctx, tc, outs, ins, slot_nmb)
    nc.compile()
    return nc


def host_prep(q, k, attn_mask, w_q, w_k, n_cores=N_CORES):
    """Weight folding, batch->core assignment, gather indices, input maps."""
    scale = float(E) ** -0.5
    W = (w_q.astype(np.float64).T @ w_k.astype(np.float64)) * scale
    wt = np.ascontiguousarray(W.T).astype(COMPUTE_NP)
    wkt = np.ascontiguousarray(w_k.T).astype(COMPUTE_NP)
    qbf = q.astype(COMPUTE_NP)
    kbf = k.astype(COMPUTE_NP)

    bsz = q.shape[0]
    b_loc = bsz // n_cores
    m_eff = (attn_mask != 0).sum(axis=1)
    order = np.argsort(-m_eff, kind="stable")  # descending m_eff
    # slot s of core c runs batch order[s*n_cores + c]; slot shape is the
    # max block count in each slot group = its first (largest) member.
    blocks = np.maximum(np.ceil(m_eff / P).astype(int), 1)
    slot_nmb = tuple(int(blocks[order[s * n_cores]]) for s in range(b_loc))
    nmb_tot = sum(slot_nmb)
    slot_base = [sum(slot_nmb[:s]) for s in range(b_loc)]

    in_maps = []
    perm = np.zeros((n_cores, b_loc), np.int64)
    for c in range(n_cores):
        idx = np.zeros((P, nmb_tot), np.int32)
        maskcol = np.full((nmb_tot, P), np.float32(MASK_NEG), np.float32)
        qs = []
        ks = []
        for s in range(b_loc):
            gb = int(order[s * n_cores + c])
            perm[c, s] = gb
            qs.append(qbf[gb])
            ks.append(kbf[gb])
            rows = np.nonzero(attn_mask[gb])[0].astype(np.int64)
            m_pad = slot_nmb[s] * P
            assert len(rows) <= m_pad, (gb, len(rows), m_pad)
            padded = np.zeros(m_pad, np.int64)
            padded[: len(rows)] = rows
            idx[:, slot_base[s] : slot_base[s] + slot_nmb[s]] = (
                (padded + s * NK).reshape(slot_nmb[s], P).T
            )
            maskcol.reshape(-1)[slot_base[s] * P : slot_base[s] * P + len(rows)] = 0.0
        in_maps.append(
            {
                "qb": np.ascontiguousarray(np.stack(qs)),
                "kb": np.ascontiguousarray(np.concatenate(ks, axis=0)),
                "wt": wt,
                "wkt": wkt,
                "maskcol": np.ascontiguousarray(maskcol.T),
                "idx": idx,
            }
        )
    return in_maps, perm, slot_nmb


_NC_CACHE = {}


def kernel(q, k, attn_mask, w_q, w_k, trace=False):
    q = np.asarray(q, dtype=np.float32)
    k = np.asarray(k, dtype=np.float32)
    w_q = np.asarray(w_q, dtype=np.float32)
    w_k = np.asarray(w_k, dtype=np.float32)
    attn_mask = np.asarray(attn_mask)

    in_maps, perm, slot_nmb = host_prep(q, k, attn_mask, w_q, w_k)
    if slot_nmb not in _NC_CACHE:
        _NC_CACHE[slot_nmb] = build_module(slot_nmb)
    nc = _NC_CACHE[slot_nmb]

    res = run_bass_kernel_spmd(nc, in_maps, core_ids=list(range(N_CORES)), trace=trace)
    out = np.zeros((B, NQ, E), np.float32)
    for c in range(N_CORES):
        out[perm[c]] = res.results[c]["out"].astype(np.float32)
    if trace:
        kernel.last_exec_time_ns = res.exec_time_ns
        kernel.last_results = res
    return out
